# revision 13
# baseline (speedup 1.0000x reference)
"""Trainium2 Bass kernel for nn_MACTitanLayer (MAC Titan layer, 8 cores).

Strategy (K-sharding of the dominant final_w matmul):
  - final_w [9216, 19968] contracts over xe features k=(l, h), l an encoder
    position (208), h a feature (96). Core c owns positions l in
    [26c, 26c+26), i.e. contraction rows [2496c, 2496c+2496).
  - The same position-sharding splits the expensive encoder parts
    (attention out, LN1/FFN/LN2, xe-silu) 8x.
  - Each core computes a partial xf [768, 96], summed with one AllReduce.
  - The small TTT tail (neural-memory grad step + re-retrieve) is
    replicated on every core after the AllReduce.
Activations are feature-major [feat, token]; per-token reductions
(l2norm/LN) use ones-matmul partition sums + ones-outer broadcasts; grad
matmuls contracting over tokens use PE transposes. All partition bases are
kept 0/32/64/96-aligned (heads live on free axes).
"""

import math

import numpy as np
import ml_dtypes

import concourse.bass as bass
import concourse.mybir as mybir
import concourse.tile as tile
from concourse import bacc
from concourse import bass_utils
from concourse.bass import ds
from concourse.masks import make_identity

F32 = mybir.dt.float32
F32R = mybir.dt.float32r
BF16 = mybir.dt.bfloat16
AF = mybir.ActivationFunctionType
OP = mybir.AluOpType

B, S, H, PM, FF, NH = 8, 96, 96, 16, 2048, 2
ALPHA, THETA = 0.999, 0.3
L = PM + 2 * S            # 208 encoder tokens per batch
NC = 8
LSH = L // NC             # 26 positions per core
DK = LSH * H              # 2496 contraction rows per core
DOUT = S * H              # 9216
TQ = B * S                # 768 query-path tokens
HD = H // NH              # 48
NTOK = B * L              # 1664
TSH = B * LSH             # 208 sharded tokens per core
CH = TQ // 2              # 384
NT = TQ // 128            # 6 token tiles
NTC = NT // 2             # 3 token tiles per chunk

CFG = {
    "w_dtype": "f32",      # final_w stream dtype: "f32r" | "f32" | "bf16"
    "w_bufs": 13,
    "oc": 768,             # big-matmul output chunk
    "ll2": 2,              # ll positions per weight DMA
}

_CACHE = {}


def _mm(nc, out, lhsT, rhs, start, stop, f32r=False):
    if f32r and lhsT.dtype == F32:
        lhsT = lhsT.bitcast(F32R)
        rhs = rhs.bitcast(F32R)
    nc.tensor.matmul(out, lhsT, rhs, start=start, stop=stop)


def build(cfg):
    nc = bacc.Bacc("TRN2", target_bir_lowering=False, debug=False, num_devices=NC)
    wdt = {"f32r": F32, "f32": F32, "bf16": BF16}[cfg["w_dtype"]]

    def din(name, shape, dt=F32):
        return nc.dram_tensor(name, shape, dt, kind="ExternalInput")

    dd = dict(
        xT_d=din("xT", [H, TQ]),
        pmT_d=din("pmT", [H, PM]),
        qwT_d=din("qwT", [H, H]),
        qb_d=din("qb", [H, 1]),
        ipqT_d=din("ipqT", [H, NH, HD]),   # per-head q proj (pre-scaled)
        ipkT_d=din("ipkT", [H, NH, HD]),
        ipvT_d=din("ipvT", [H, H]),
        ipqb_d=din("ipqb", [HD, NH, 1]),
        ipkb_d=din("ipkb", [HD, NH, 1]),
        ipvb_d=din("ipvb", [1, H]),
        opT_d=din("opT", [HD, NH, H]),     # out_proj.T split by head k-tiles
        opb_d=din("opb", [H, 1]),
        ln1w_d=din("ln1w", [H, 1]), ln1b_d=din("ln1b", [H, 1]),
        ln2w_d=din("ln2w", [H, 1]), ln2b_d=din("ln2b", [H, 1]),
        f1T_d=din("f1T", [H, FF]),
        f1b_d=din("f1b", [128, FF // 128, 1]),
        f2T_d=din("f2T", [128, FF // 128, H]),
        f2b_d=din("f2b", [H, 1]),
        kwT_d=din("kwT", [H, H]), kb_d=din("kb", [H, 1]),
        vwT_d=din("vwT", [H, H]), vb_d=din("vb", [H, 1]),
        m1T_d=din("m1T", [H, 2 * H]),
        m1b_d=din("m1b", [H, 2, 1]),
        m2T_d=din("m2T", [H, 2, H]),       # m2_w.T k-tiles
        m2b_d=din("m2b", [H, 1]),
        m2w_d=din("m2w", [H, 2 * H]),
        fb768_d=din("fb768", [128, NT, H]),
        wt_d=din("WTc", [DK, DOUT], wdt),
    )
    dd["out_d"] = nc.dram_tensor("outf", [H, TQ], F32, kind="ExternalOutput")
    if cfg.get("debug"):
        for nm, shp in [("d_xcf", [H, B, L]), ("d_qsel", [HD, NH, B, LSH]),
                        ("d_kf", [HD, NH, B, L]), ("d_of", [HD, NH, B, LSH]),
                        ("d_x1", [H, B, LSH]), ("d_x1n", [H, TSH]),
                        ("d_x2", [H, TSH]), ("d_xef", [H, TSH]),
                        ("d_arin", [TQ, H]), ("d_arout", [TQ, H]),
                        ("d_nm1T", [H, 2 * H]), ("d_nm2T", [H, 2, H]),
                        ("d_q2", [H, TQ]), ("d_vtm0", [128, B, H])]:
            dd[nm] = nc.dram_tensor(nm, shp, F32, kind="ExternalOutput")

    with tile.TileContext(nc) as tc:
        _body(nc, tc, dd, cfg, wdt)
    nc.compile()
    return nc


def _body(nc, tc, dd, cfg, wdt):
    w_f32r = cfg["w_dtype"] == "f32r"
    OC = cfg["oc"]
    LL2 = cfg["ll2"]

    from contextlib import ExitStack
    stack = ExitStack()

    def pool(name, bufs, space="SBUF"):
        return stack.enter_context(tc.tile_pool(name=name, bufs=bufs, space=space))

    const = pool("const", 1)
    big = pool("big", 1)
    work = pool("work", 1)
    wstr = pool("wstr", cfg["w_bufs"])
    pss = pool("pss", 4, "PSUM")
    psb = pool("psb", 2, "PSUM")
    dram = pool("dram", 1, "DRAM")

    def ld(dram_t, tag):
        t = const.tile(list(dram_t.shape), dram_t.dtype, tag=tag, name=tag)
        nc.sync.dma_start(t[:], dram_t[:])
        return t

    qwT = ld(dd["qwT_d"], "qwT"); qb = ld(dd["qb_d"], "qb")
    ipqT = ld(dd["ipqT_d"], "ipqT"); ipkT = ld(dd["ipkT_d"], "ipkT")
    ipvT = ld(dd["ipvT_d"], "ipvT")
    ipqb = ld(dd["ipqb_d"], "ipqb"); ipkb = ld(dd["ipkb_d"], "ipkb")
    opT = ld(dd["opT_d"], "opT"); opb = ld(dd["opb_d"], "opb")
    ln1w = ld(dd["ln1w_d"], "ln1w"); ln1b = ld(dd["ln1b_d"], "ln1b")
    ln2w = ld(dd["ln2w_d"], "ln2w"); ln2b = ld(dd["ln2b_d"], "ln2b")
    f1T = ld(dd["f1T_d"], "f1T"); f1b = ld(dd["f1b_d"], "f1b")
    f2T = ld(dd["f2T_d"], "f2T"); f2b = ld(dd["f2b_d"], "f2b")
    kwT = ld(dd["kwT_d"], "kwT"); kb = ld(dd["kb_d"], "kb")
    vwT = ld(dd["vwT_d"], "vwT"); vb = ld(dd["vb_d"], "vb")
    m1T = ld(dd["m1T_d"], "m1T"); m1b = ld(dd["m1b_d"], "m1b")
    m2T = ld(dd["m2T_d"], "m2T"); m2b = ld(dd["m2b_d"], "m2b")
    m2w = ld(dd["m2w_d"], "m2w")
    pmT = ld(dd["pmT_d"], "pmT")
    fb768 = ld(dd["fb768_d"], "fb768")

    vb_bc = const.tile([128, H], F32, tag="vb_bc", name="vb_bc")
    nc.sync.dma_start(vb_bc[:], dd["ipvb_d"][:].to_broadcast([128, H]))

    ident = const.tile([128, 128], F32, tag="ident", name="ident")
    make_identity(nc, ident[:])
    ones_col = const.tile([H, 1], F32, tag="ones_col", name="ones_col")
    nc.vector.memset(ones_col[:], 1.0)
    ones_row = const.tile([1, H], F32, tag="ones_row", name="ones_row")
    nc.vector.memset(ones_row[:], 1.0)
    zb = const.tile([128, 1], F32, tag="zb", name="zb")
    nc.vector.memset(zb[:], 0.0)
    eps1 = const.tile([1, 1], F32, tag="eps1", name="eps1")
    nc.vector.memset(eps1[:], 1e-5)

    pid = nc.partition_id()
    qoff = pid * LSH

    # ============ F0: shared front ============
    xT = big.tile([H, TQ], F32, tag="xT", name="xT")
    nc.sync.dma_start(xT[:], dd["xT_d"][:])

    xcf = big.tile([H, B, L], F32, tag="xcf", name="xcf")
    nc.vector.tensor_copy(xcf[:, :, 0:PM],
                          pmT[:].unsqueeze(1).to_broadcast([H, B, PM]))
    nc.vector.tensor_copy(xcf[:, :, PM + S:L],
                          xT[:].rearrange("h (b s) -> h b s", b=B))

    # queries -> neural-memory retrieve -> nmm, chunk-wise over 384 tokens
    for c in range(2):
        sl = slice(c * CH, (c + 1) * CH)
        ps = pss.tile([H, CH], F32, tag="ps", name="ps_q1")
        _mm(nc, ps[:], qwT[:], xT[:, sl], True, True)
        q1c = work.tile([H, CH], F32, tag="q1c", name="q1c")
        nc.scalar.activation(q1c[:], ps[:], AF.Identity, bias=qb[:])
        qryc = work.tile([H, CH], F32, tag="qryc", name="qryc")
        _l2norm_fm(nc, pss, work, q1c, qryc, ones_col, ones_row, zb, silu=True)
        h1 = []
        for m in range(2):
            psm = pss.tile([H, CH], F32, tag="ps", name="ps_h1")
            _mm(nc, psm[:], m1T[:, m * H:(m + 1) * H], qryc[:], True, True)
            h1c = work.tile([H, CH], F32, tag="h1c", name="h1c", bufs=2)
            nc.scalar.activation(h1c[:], psm[:], AF.Silu, bias=m1b[:, m, :])
            h1.append(h1c)
        ps2 = pss.tile([H, CH], F32, tag="ps", name="ps_nmm")
        _mm(nc, ps2[:], m2T[:, 0, :], h1[0][:], True, False)
        _mm(nc, ps2[:], m2T[:, 1, :], h1[1][:], False, True)
        # nmm chunk c covers batches 4c..4c+4, all s
        nc.scalar.activation(
            xcf[:, c * 4:(c + 1) * 4, PM:PM + S],
            ps2[:].rearrange("h (b s) -> h b s", b=4), AF.Identity, bias=m2b[:])

    # k projection (all tokens) + q projection (only my 26 positions/batch)
    kf = big.tile([HD, NH, B, L], F32, tag="kf", name="kf")
    q_sel = big.tile([HD, NH, B, LSH], F32, tag="q_sel", name="q_sel")
    xcf_flat = xcf[:].rearrange("h b l -> h (b l)")
    ECH = NTOK // 4
    for c in range(4):
        sl = slice(c * ECH, (c + 1) * ECH)
        for hh in range(NH):
            ps = pss.tile([HD, ECH], F32, tag="ps", name="ps_qkv")
            _mm(nc, ps[:], ipkT[:, hh, :], xcf_flat[:, sl], True, True)
            nc.scalar.activation(
                kf[:].rearrange("d n b l -> d n (b l)")[:, hh, sl],
                ps[:], AF.Identity, bias=ipkb[:, hh, :])
    for hh in range(NH):
        ps = pss.tile([HD, TSH], F32, tag="ps", name="ps_qp")
        _mm(nc, ps[:], ipqT[:, hh, :], xcf[:, :, ds(qoff, LSH)], True, True)
        nc.scalar.activation(q_sel[:, hh, :, :],
                             ps[:].rearrange("d (b l) -> d b l", b=B),
                             AF.Identity, bias=ipqb[:, hh, :])

    # v token-major per batch: [128+80, B, H]
    v_tm0 = big.tile([128, B, H], F32, tag="v_tm0", name="v_tm0")
    v_tm1 = big.tile([80, B, H], F32, tag="v_tm1", name="v_tm1")
    for b in range(B):
        for tt, dst, npart in ((0, v_tm0, 128), (1, v_tm1, 80)):
            ps = pss.tile([128, H], F32, tag="ps", name="ps_v")
            toks = slice(b * L + tt * 128, b * L + tt * 128 + npart)
            _mm(nc, ps[:npart, :], xcf_flat[:, toks], ipvT[:], True, True)
            nc.vector.tensor_add(dst[:, b, :], ps[:npart, :], vb_bc[:npart, :])

    # ============ F1: attention + encoder (my 26 positions) ============
    of = big.tile([HD, NH, B, LSH], F32, tag="of", name="of")
    for b in range(B):
        for hh in range(NH):
            ps_s = pss.tile([LSH, L], F32, tag="ps", name="ps_s")
            _mm(nc, ps_s[:], q_sel[:, hh, b, :], kf[:, hh, b, :],
                True, True)
            e = work.tile([LSH, L], F32, tag="sm_e", name="sm_e")
            den = work.tile([LSH, 1], F32, tag="sm_d", name="sm_d")
            nc.scalar.activation(e[:], ps_s[:], AF.Exp, bias=zb[:LSH, :], accum_out=den[:])
            rden = work.tile([LSH, 1], F32, tag="sm_r", name="sm_r")
            nc.vector.reciprocal(rden[:], den[:])
            a = work.tile([LSH, L], F32, tag="sm_a", name="sm_a")
            nc.vector.tensor_scalar_mul(a[:], e[:], rden[:])
            ps_o = pss.tile([HD, LSH], F32, tag="ps", name="ps_o")
            for tt, vsrc, npart in ((0, v_tm0, 128), (1, v_tm1, 80)):
                ps_t = pss.tile([128, LSH], F32, tag="ps", name="ps_t")
                nc.tensor.transpose(ps_t[:npart, :],
                                    a[:, tt * 128:tt * 128 + npart],
                                    ident[:LSH, :LSH])
                at = work.tile([128, LSH], F32, tag="at", name="at")
                nc.vector.tensor_copy(at[:npart, :], ps_t[:npart, :])
                _mm(nc, ps_o[:], vsrc[:, b, hh * HD:(hh + 1) * HD],
                    at[:npart, :], tt == 0, tt == 1)
            nc.vector.tensor_copy(of[:, hh, b, :], ps_o[:])

    # out_proj (2 head k-tiles) + residual
    ps = pss.tile([H, TSH], F32, tag="ps", name="ps_op")
    for hh in range(NH):
        _mm(nc, ps[:], opT[:, hh, :],
            of[:, hh, :, :].rearrange("d b l -> d (b l)"), hh == 0, hh == 1)
    x1 = big.tile([H, B, LSH], F32, tag="x1", name="x1")
    tmp = work.tile([H, TSH], F32, tag="w208", name="tmp_op")
    nc.vector.tensor_scalar_add(tmp[:], ps[:], opb[:])
    nc.vector.tensor_add(x1[:], tmp[:].rearrange("h (b l) -> h b l", b=B),
                         xcf[:, :, ds(qoff, LSH)])
    x1f = x1[:].rearrange("h b l -> h (b l)")

    x1n = big.tile([H, TSH], F32, tag="x1n", name="x1n")
    _layernorm_fm(nc, pss, work, x1f, x1n[:], ln1w, ln1b, ones_col, ones_row, zb, eps1)

    ps2 = pss.tile([H, TSH], F32, tag="ps", name="ps_ff2")
    for m in range(FF // 128):
        psf = pss.tile([128, TSH], F32, tag="ps", name="ps_ff1")
        _mm(nc, psf[:], f1T[:, m * 128:(m + 1) * 128], x1n[:], True, True)
        h_ffn = work.tile([128, TSH], F32, tag="h_ffn", name="h_ffn", bufs=3)
        nc.scalar.activation(h_ffn[:], psf[:], AF.Silu, bias=f1b[:, m, :])
        _mm(nc, ps2[:], f2T[:, m, :], h_ffn[:], m == 0, m == FF // 128 - 1)
    x2 = big.tile([H, TSH], F32, tag="x2", name="x2")
    tmp2 = work.tile([H, TSH], F32, tag="w208", name="tmp_ff")
    nc.vector.tensor_scalar_add(tmp2[:], ps2[:], f2b[:])
    nc.vector.tensor_add(x2[:], tmp2[:], x1n[:])

    e2 = big.tile([H, TSH], F32, tag="e2", name="e2")
    _layernorm_fm(nc, pss, work, x2[:], e2[:], ln2w, ln2b, ones_col, ones_row, zb, eps1)
    xef = big.tile([H, TSH], F32, tag="xef", name="xef")
    nc.scalar.activation(xef[:], e2[:], AF.Silu, bias=zb[:H, :])
    if wdt == BF16:
        xef_mm = big.tile([H, TSH], BF16, tag="xef_bf", name="xef_bf")
        nc.vector.tensor_copy(xef_mm[:], xef[:])
    else:
        xef_mm = xef
    xe3 = xef_mm[:].rearrange("h (b l) -> h b l", b=B)

    if cfg.get("debug"):
        nc.sync.dma_start(dd["d_xcf"][:], xcf[:])
        nc.sync.dma_start(dd["d_qsel"][:], q_sel[:])
        nc.sync.dma_start(dd["d_kf"][:], kf[:])
        nc.sync.dma_start(dd["d_of"][:], of[:])
        nc.sync.dma_start(dd["d_x1"][:], x1[:])
        nc.sync.dma_start(dd["d_x1n"][:], x1n[:])
        nc.sync.dma_start(dd["d_x2"][:], x2[:])
        nc.sync.dma_start(dd["d_xef"][:], xef[:])
        nc.sync.dma_start(dd["d_vtm0"][:], v_tm0[:])

    # ============ F2: big matmul (K-sharded) ============
    ar_in = dram.tile([TQ, H], F32, tag="ar_in", name="ar_in")
    ar_out = dram.tile([TQ, H], F32, tag="ar_out", name="ar_out")
    ar_in3 = ar_in[:].rearrange("(b s) h -> b s h", b=B)
    wt3 = dd["wt_d"][:].rearrange("(l h) o -> h l o", h=H)
    n_oc = DOUT // OC
    SROWS = OC // H
    for ci in range(n_oc):
        psx = psb.tile([B, OC], F32, tag="ps_big", name="psx")
        for l0 in range(0, LSH, LL2):
            wt = wstr.tile([H, LL2, OC], wdt, tag="wt", name="wt")
            nc.sync.dma_start(wt[:], wt3[:, l0:l0 + LL2, ci * OC:(ci + 1) * OC])
            for l1 in range(LL2):
                ll = l0 + l1
                for j0 in range(0, OC, 512):
                    j1 = min(j0 + 512, OC)
                    _mm(nc, psx[:, j0:j1], xe3[:, :, ll], wt[:, l1, j0:j1],
                        ll == 0, ll == LSH - 1, f32r=w_f32r)
        xfp = work.tile([B, OC], F32, tag="xfp", name="xfp", bufs=2)
        nc.scalar.copy(xfp[:], psx[:])
        nc.sync.dma_start(ar_in3[:, ci * SROWS:(ci + 1) * SROWS, :],
                          xfp[:].rearrange("b (s h) -> b s h", h=H))

    nc.gpsimd.collective_compute(
        "AllReduce", OP.add,
        replica_groups=[list(range(NC))],
        ins=[ar_in[:].opt()],
        outs=[ar_out[:].opt()],
    )

    if cfg.get("debug"):
        nc.sync.dma_start(dd["d_arin"][:], ar_in[:])
        nc.sync.dma_start(dd["d_arout"][:], ar_out[:])

    # ============ T: tail (replicated) ============
    xf_tm = big.tile([128, NT, H], F32, tag="xcf", name="xf_tm")
    nc.sync.dma_start(xf_tm[:], ar_out[:].rearrange("(t p) h -> p t h", p=128))
    nc.vector.tensor_add(xf_tm[:], xf_tm[:], fb768[:])
    xff = big.tile([H, TQ], F32, tag="xT", name="xff")
    for t in range(NT):
        ps_t = pss.tile([H, 128], F32, tag="ps", name="ps_xf")
        nc.tensor.transpose(ps_t[:], xf_tm[:, t, :], ident[:])
        nc.vector.tensor_copy(xff[:, t * 128:(t + 1) * 128], ps_t[:])

    kp_tm = big.tile([128, NT, H], F32, tag="kp_tm", name="kp_tm")
    dpred_tm = big.tile([128, NT, H], F32, tag="dpred_tm", name="dpred_tm")
    dz_tm = [big.tile([128, NT, H], F32, tag="dz_tm_a", name="dz_tm_a"),
             big.tile([128, NT, H], F32, tag="kf", name="dz_tm_b")]
    h_tm = [big.tile([128, NT, H], F32, tag="v_tm0", name="h_tm_a"),
            big.tile([128, NT, H], F32, tag="v_tm1", name="h_tm_b")]
    gb1 = [work.tile([H, 1], F32, tag=f"gb1_{m}", name=f"gb1_{m}", bufs=1)
           for m in range(2)]
    gb2 = work.tile([H, 1], F32, tag="gb2", name="gb2", bufs=1)

    def transpose_chunk(src_ap, dst_tiles, c):
        """src [96, 384] -> dst token tiles 3c..3c+3 ([128, t, 96])"""
        for t in range(NTC):
            ps_t = pss.tile([128, H], F32, tag="ps", name="ps_tm")
            nc.tensor.transpose(ps_t[:], src_ap[:, t * 128:(t + 1) * 128],
                                ident[:H, :H])
            nc.vector.tensor_copy(dst_tiles[:, c * NTC + t, :], ps_t[:])

    for c in range(2):
        sl = slice(c * CH, (c + 1) * CH)
        ps_k = pss.tile([H, CH], F32, tag="ps", name="ps_kp")
        _mm(nc, ps_k[:], kwT[:], xff[:, sl], True, True)
        kp_c = work.tile([H, CH], F32, tag="kp_c", name="kp_c")
        nc.scalar.activation(kp_c[:], ps_k[:], AF.Identity, bias=kb[:])
        transpose_chunk(kp_c[:], kp_tm, c)
        ps_v = pss.tile([H, CH], F32, tag="ps", name="ps_vp")
        _mm(nc, ps_v[:], vwT[:], xff[:, sl], True, True)
        vp_c = work.tile([H, CH], F32, tag="vp_c", name="vp_c")
        nc.scalar.activation(vp_c[:], ps_v[:], AF.Identity, bias=vb[:])

        hs = []
        sp = []
        for m in range(2):
            ps_z = pss.tile([H, CH], F32, tag="ps", name="ps_z")
            _mm(nc, ps_z[:], m1T[:, m * H:(m + 1) * H], kp_c[:], True, True)
            z_m = work.tile([H, CH], F32, tag="z_m", name="z_m", bufs=2)
            nc.scalar.activation(z_m[:], ps_z[:], AF.Identity, bias=m1b[:, m, :])
            sg_m = work.tile([H, CH], F32, tag="sg_m", name="sg_m", bufs=2)
            nc.scalar.activation(sg_m[:], z_m[:], AF.Sigmoid, bias=zb[:H, :])
            h_m = work.tile([H, CH], F32, tag="h_m", name="h_m", bufs=2)
            nc.vector.tensor_mul(h_m[:], z_m[:], sg_m[:])
            transpose_chunk(h_m[:], h_tm[m], c)
            t1 = work.tile([H, CH], F32, tag="t1_m", name="t1_m")
            nc.vector.tensor_sub(t1[:], z_m[:], h_m[:])
            nc.vector.tensor_scalar_add(t1[:], t1[:], 1.0)
            sp_m = work.tile([H, CH], F32, tag="sp_m", name="sp_m", bufs=2)
            nc.vector.tensor_mul(sp_m[:], sg_m[:], t1[:])
            hs.append(h_m)
            sp.append(sp_m)

        ps_p = pss.tile([H, CH], F32, tag="ps", name="ps_pred")
        _mm(nc, ps_p[:], m2T[:, 0, :], hs[0][:], True, False)
        _mm(nc, ps_p[:], m2T[:, 1, :], hs[1][:], False, True)
        pr = work.tile([H, CH], F32, tag="pr_c", name="pr_c")
        nc.scalar.activation(pr[:], ps_p[:], AF.Identity, bias=m2b[:])
        dpr = work.tile([H, CH], F32, tag="dpr_c", name="dpr_c")
        nc.vector.tensor_sub(dpr[:], pr[:], vp_c[:])
        nc.vector.tensor_scalar_mul(dpr[:], dpr[:], 2.0 / (TQ * H))
        transpose_chunk(dpr[:], dpred_tm, c)
        gpart = work.tile([H, 1], F32, tag="gpart", name="gpart")
        nc.vector.reduce_sum(gpart[:], dpr[:], axis=mybir.AxisListType.X)
        if c == 0:
            nc.vector.tensor_copy(gb2[:], gpart[:])
        else:
            nc.vector.tensor_add(gb2[:], gb2[:], gpart[:])

        for m in range(2):
            ps_dh = pss.tile([H, CH], F32, tag="ps", name="ps_dh")
            _mm(nc, ps_dh[:], m2w[:, m * H:(m + 1) * H], dpr[:], True, True)
            dz_m = work.tile([H, CH], F32, tag="dz_m", name="dz_m")
            nc.vector.tensor_mul(dz_m[:], ps_dh[:], sp[m][:])
            transpose_chunk(dz_m[:], dz_tm[m], c)
            gp1 = work.tile([H, 1], F32, tag="gp1", name="gp1")
            nc.vector.reduce_sum(gp1[:], dz_m[:], axis=mybir.AxisListType.X)
            if c == 0:
                nc.vector.tensor_copy(gb1[m][:], gp1[:])
            else:
                nc.vector.tensor_add(gb1[m][:], gb1[m][:], gp1[:])

    # grads -> new params
    nm1T = big.tile([H, 2 * H], F32, tag="nm1T", name="nm1T")
    for m in range(2):
        ps_g1 = pss.tile([H, H], F32, tag="ps", name="ps_g1")
        for t in range(NT):
            _mm(nc, ps_g1[:], kp_tm[:, t, :], dz_tm[m][:, t, :],
                t == 0, t == NT - 1)
        msl = slice(m * H, (m + 1) * H)
        tgw = work.tile([H, H], F32, tag="tgw", name="tgw")
        nc.vector.tensor_scalar_mul(tgw[:], ps_g1[:], THETA)
        nc.vector.tensor_scalar(nm1T[:, msl], m1T[:, msl], ALPHA, None, OP.mult)
        nc.vector.tensor_sub(nm1T[:, msl], nm1T[:, msl], tgw[:])

    nm2T = big.tile([H, 2, H], F32, tag="nm2T", name="nm2T")
    for m in range(2):
        ps_g2 = pss.tile([H, H], F32, tag="ps", name="ps_g2")
        for t in range(NT):
            _mm(nc, ps_g2[:], h_tm[m][:, t, :], dpred_tm[:, t, :],
                t == 0, t == NT - 1)
        tg2 = work.tile([H, H], F32, tag="tg2", name="tg2")
        nc.vector.tensor_scalar_mul(tg2[:], ps_g2[:], THETA)
        nc.vector.tensor_scalar(nm2T[:, m, :], m2T[:, m, :], ALPHA, None, OP.mult)
        nc.vector.tensor_sub(nm2T[:, m, :], nm2T[:, m, :], tg2[:])

    nm1b = big.tile([H, 2, 1], F32, tag="nm1b", name="nm1b")
    nm2b = big.tile([H, 1], F32, tag="nm2b", name="nm2b")
    for m in range(2):
        nc.vector.tensor_scalar_mul(gb1[m][:], gb1[m][:], THETA)
        nc.vector.tensor_scalar(nm1b[:, m, :], m1b[:, m, :], ALPHA, None, OP.mult)
        nc.vector.tensor_sub(nm1b[:, m, :], nm1b[:, m, :], gb1[m][:])
    nc.vector.tensor_scalar_mul(gb2[:], gb2[:], THETA)
    nc.vector.tensor_scalar(nm2b[:], m2b[:], ALPHA, None, OP.mult)
    nc.vector.tensor_sub(nm2b[:], nm2b[:], gb2[:])

    if cfg.get("debug"):
        nc.sync.dma_start(dd["d_nm1T"][:], nm1T[:])
        nc.sync.dma_start(dd["d_nm2T"][:], nm2T[:])

    # retrieve with updated memory; out = xf * sigmoid(y)
    for c in range(2):
        sl = slice(c * CH, (c + 1) * CH)
        ps_q = pss.tile([H, CH], F32, tag="ps", name="ps_q2")
        _mm(nc, ps_q[:], qwT[:], xff[:, sl], True, True)
        q2r = work.tile([H, CH], F32, tag="q2r", name="q2r")
        nc.scalar.activation(q2r[:], ps_q[:], AF.Identity, bias=qb[:])
        q2 = work.tile([H, CH], F32, tag="q2", name="q2")
        _l2norm_fm(nc, pss, work, q2r, q2, ones_col, ones_row, zb, silu=False)
        if cfg.get("debug"):
            nc.sync.dma_start(dd["d_q2"][:, sl], q2[:])
        uu = []
        for m in range(2):
            ps_u = pss.tile([H, CH], F32, tag="ps", name="ps_u")
            _mm(nc, ps_u[:], nm1T[:, m * H:(m + 1) * H], q2[:], True, True)
            u_m = work.tile([H, CH], F32, tag="u_m", name="u_m", bufs=2)
            nc.scalar.activation(u_m[:], ps_u[:], AF.Silu, bias=nm1b[:, m, :])
            uu.append(u_m)
        ps_y = pss.tile([H, CH], F32, tag="ps", name="ps_y")
        _mm(nc, ps_y[:], nm2T[:, 0, :], uu[0][:], True, False)
        _mm(nc, ps_y[:], nm2T[:, 1, :], uu[1][:], False, True)
        sg_c = work.tile([H, CH], F32, tag="sg_c", name="sg_c")
        nc.scalar.activation(sg_c[:], ps_y[:], AF.Sigmoid, bias=nm2b[:])
        ot = work.tile([H, CH], F32, tag="ot", name="ot")
        nc.vector.tensor_mul(ot[:], xff[:, sl], sg_c[:])
        nc.sync.dma_start(dd["out_d"][:, sl], ot[:])

    stack.close()


def _l2norm_fm(nc, pss, work, src, dst, ones_col, ones_row, zb, silu):
    """dst = (silu?)(src / max(||src||_partcol, 1e-12)); src/dst [96, T] tiles."""
    T = src.shape[1]
    ps = pss.tile([1, T], F32, tag="ps", name="ps_l2s")
    sq = work.tile([H, T], F32, tag="l2_sq", name="l2_sq")
    nc.vector.tensor_mul(sq[:], src[:], src[:])
    _mm(nc, ps[:], ones_col[:], sq[:], True, True)
    nrm = work.tile([1, T], F32, tag="l2_nrm", name="l2_nrm")
    nc.scalar.activation(nrm[:], ps[:], AF.Sqrt, bias=zb[:1, :])
    nc.vector.tensor_scalar_max(nrm[:], nrm[:], 1e-12)
    inv = work.tile([1, T], F32, tag="l2_inv", name="l2_inv")
    nc.vector.reciprocal(inv[:], nrm[:])
    psb_ = pss.tile([H, T], F32, tag="ps", name="ps_l2b")
    _mm(nc, psb_[:], ones_row[:], inv[:], True, True)
    if silu:
        tmp = work.tile([H, T], F32, tag="l2_tmp", name="l2_tmp")
        nc.vector.tensor_mul(tmp[:], src[:], psb_[:])
        nc.scalar.activation(dst[:], tmp[:], AF.Silu, bias=zb[:H, :])
    else:
        nc.vector.tensor_mul(dst[:], src[:], psb_[:])


def _layernorm_fm(nc, pss, work, src_ap, dst_ap, w_ap, b_ap, ones_col, ones_row, zb, eps1):
    """dst = LN(src) * w + b over the feature (partition) axis; [96, T] APs."""
    T = src_ap.shape[-1]
    ps_s = pss.tile([1, T], F32, tag="ps", name="ps_lns")
    _mm(nc, ps_s[:], ones_col[:], src_ap, True, True)
    mean = work.tile([1, T], F32, tag="ln_mean", name="ln_mean")
    nc.scalar.activation(mean[:], ps_s[:], AF.Identity, bias=zb[:1, :], scale=1.0 / H)
    sq = work.tile([H, T], F32, tag="ln_sq", name="ln_sq")
    nc.vector.tensor_mul(sq[:], src_ap, src_ap)
    ps_q = pss.tile([1, T], F32, tag="ps", name="ps_lnq")
    _mm(nc, ps_q[:], ones_col[:], sq[:], True, True)
    var = work.tile([1, T], F32, tag="ln_var", name="ln_var")
    nc.scalar.activation(var[:], ps_q[:], AF.Identity, bias=zb[:1, :], scale=1.0 / H)
    m2t = work.tile([1, T], F32, tag="ln_m2", name="ln_m2")
    nc.vector.tensor_mul(m2t[:], mean[:], mean[:])
    nc.vector.tensor_sub(var[:], var[:], m2t[:])
    sd = work.tile([1, T], F32, tag="ln_sd", name="ln_sd")
    nc.scalar.activation(sd[:], var[:], AF.Sqrt, bias=eps1[:])
    rstd = work.tile([1, T], F32, tag="ln_rstd", name="ln_rstd")
    nc.vector.reciprocal(rstd[:], sd[:])
    nmr = work.tile([1, T], F32, tag="ln_nmr", name="ln_nmr")
    nc.vector.tensor_mul(nmr[:], mean[:], rstd[:])
    nc.vector.tensor_scalar_mul(nmr[:], nmr[:], -1.0)
    ps_a = pss.tile([H, T], F32, tag="ps", name="ps_lna")
    _mm(nc, ps_a[:], ones_row[:], rstd[:], True, True)
    ps_c = pss.tile([H, T], F32, tag="ps", name="ps_lnc")
    _mm(nc, ps_c[:], ones_row[:], nmr[:], True, True)
    t1 = work.tile([H, T], F32, tag="ln_t1", name="ln_t1")
    nc.vector.tensor_mul(t1[:], src_ap, ps_a[:])
    nc.vector.tensor_add(t1[:], t1[:], ps_c[:])
    nc.vector.tensor_scalar(dst_ap, t1[:], w_ap[:], b_ap[:], OP.mult, OP.add)


def prep_inmaps(inputs, cfg=None):
    cfg = cfg or CFG
    f32 = np.float32
    wnp = {"f32r": f32, "f32": f32, "bf16": ml_dtypes.bfloat16}[cfg["w_dtype"]]

    def T(a):
        return np.ascontiguousarray(np.asarray(a, f32).T)

    x = np.asarray(inputs["x"], f32)
    ipw = np.asarray(inputs["in_proj_w"], f32)   # [288, 96]
    ipb = np.asarray(inputs["in_proj_b"], f32)   # [288]
    sc = 1.0 / math.sqrt(HD)
    qw_part = ipw[0:H] * sc                      # [96, 96]
    qb_part = ipb[0:H] * sc
    kw_part = ipw[H:2 * H]
    kb_part = ipb[H:2 * H]
    vw_part = ipw[2 * H:3 * H]
    vb_part = ipb[2 * H:3 * H]

    # per-head: ipqT [96(in), NH, 48(dout)] ; head h = rows 48h..48h+48
    ipqT = np.ascontiguousarray(qw_part.T.reshape(H, NH, HD))
    ipkT = np.ascontiguousarray(kw_part.T.reshape(H, NH, HD))
    ipqb = np.ascontiguousarray(qb_part.reshape(NH, HD).T.reshape(HD, NH, 1))
    ipkb = np.ascontiguousarray(kb_part.reshape(NH, HD).T.reshape(HD, NH, 1))

    opw = np.asarray(inputs["out_proj_w"], f32)  # [96, 96]
    # opT [48, NH, 96]: k-tile hh = in-features 48hh..48hh+48 of out_proj.T
    opT = np.ascontiguousarray(opw.T.reshape(NH, HD, H).transpose(1, 0, 2))

    f1b = np.asarray(inputs["ff1_b"], f32).reshape(FF // 128, 128, 1)
    f1b = np.ascontiguousarray(f1b.transpose(1, 0, 2))
    f2T = T(inputs["ff2_w"])                     # [2048, 96]
    f2T = np.ascontiguousarray(f2T.reshape(FF // 128, 128, H).transpose(1, 0, 2))

    m1b = np.ascontiguousarray(
        np.asarray(inputs["m1_b"], f32).reshape(2, H, 1).transpose(1, 0, 2))
    m2T = np.ascontiguousarray(
        T(inputs["m2_w"]).reshape(2, H, H).transpose(1, 0, 2))  # [96, 2, 96]

    fwT = np.ascontiguousarray(np.asarray(inputs["final_w"], f32).T)
    fbt = np.tile(np.asarray(inputs["final_b"], f32).reshape(S, H), (B, 1))
    fb768 = np.ascontiguousarray(fbt.reshape(NT, 128, H).transpose(1, 0, 2))

    col = lambda k: np.ascontiguousarray(np.asarray(inputs[k], f32).reshape(-1, 1))
    base = dict(
        xT=T(x.reshape(TQ, H)),
        pmT=T(inputs["persistent_memory"]),
        qwT=T(inputs["q_w"]), qb=col("q_b"),
        ipqT=ipqT, ipkT=ipkT, ipvT=np.ascontiguousarray(vw_part.T),
        ipqb=ipqb, ipkb=ipkb,
        ipvb=np.ascontiguousarray(vb_part.reshape(1, H)),
        opT=opT, opb=col("out_proj_b"),
        ln1w=col("ln1_w"), ln1b=col("ln1_b"),
        ln2w=col("ln2_w"), ln2b=col("ln2_b"),
        f1T=T(inputs["ff1_w"]), f1b=f1b,
        f2T=f2T, f2b=col("ff2_b"),
        kwT=T(inputs["k_w"]), kb=col("k_b"),
        vwT=T(inputs["v_w"]), vb=col("v_b"),
        m1T=T(inputs["m1_w"]), m1b=m1b,
        m2T=m2T, m2b=col("m2_b"),
        m2w=np.ascontiguousarray(np.asarray(inputs["m2_w"], f32)),
        fb768=fb768,
    )
    in_maps = []
    for c in range(NC):
        m = dict(base)
        m["WTc"] = np.ascontiguousarray(fwT[c * DK:(c + 1) * DK].astype(wnp))
        in_maps.append(m)
    return in_maps


def get_nc(cfg=None):
    cfg = cfg or CFG
    key = tuple(sorted((k, str(v)) for k, v in cfg.items()))
    if key not in _CACHE:
        _CACHE[key] = build(cfg)
    return _CACHE[key]


def kernel(**inputs):
    nc = get_nc()
    in_maps = prep_inmaps(inputs)
    res = bass_utils.run_bass_kernel_spmd(
        nc, in_maps, core_ids=list(range(NC)), trace=False
    )
    outf = res.results[0]["outf"]  # [96, 768]
    return np.ascontiguousarray(outf.T).reshape(B, S, H)


if __name__ == "__main__":
    print("building...")
    get_nc()
    print("built")
